# revision 25
# baseline (speedup 1.0000x reference)
"""AdaptDHM MoE-routing kernel for one TRN2 chip (8 NeuronCores).

Strategy (expert-parallel dispatch, done host-side):
  - router = argmax(x @ center.T) picks one of C=8 clusters per token.
  - The reference computes ALL 8 cluster towers for every token and then
    gathers the selected one; only 1/8 of that work is observable. We
    dispatch each token to the core owning its cluster and run the
    4-layer MLP (1024->2048->1024->512->1, relu/sigmoid) once per token.
  - Core d receives the tokens routed to cluster d, padded to a common
    capacity K (SPMD: all cores run the same NEFF), plus the gated
    weights w0_l * wc_l[d] in a DMA-friendly blocked layout.
  - Compute: layers 0-2 in fp8-e4m3 with DoubleRow matmuls (2x TensorE
    rate), layer 3 in bf16; fp32 PSUM accumulation throughout. Inputs and
    weights are pre-scaled into fp8's normal range; the inverse scales are
    folded into the relu/copy that writes each layer's activations.
  - On-device: feature-major layout ([feature, token]), weights stationary,
    activations moving, relu split across Scalar+Vector engines.
  - Host scatters per-core results back to the [B] output.
"""

import math
import os

import ml_dtypes
import numpy as np

B, DIMS = 8192, 1024
FCN = [DIMS, 2048, 1024, 512, 1]
C = 8
NCORES = 8
P = 128
TT = 512  # max token tile (matmul moving free dim / PSUM bank)

_BF16 = ml_dtypes.bfloat16

_graph_cache = {}
last_run = None  # BassKernelResults of the most recent kernel() call

# per-layer (in_blocks, out_blocks)
_LAYER_BLOCKS = [(8, 16), (16, 8), (8, 4), (4, 1)]
# out columns per DMA-able weight block (~256KB fp8 each)
_WBLK_OCOLS = [256, 128, 512, 1]


def _token_tiles(K):
    """Split K into near-equal tiles of size <= TT (multiples of 16)."""
    assert K % 16 == 0
    nt = max(1, math.ceil(K / TT))
    units = K // 16
    base = units // nt
    tiles = []
    t0 = 0
    for i in range(nt):
        u = base + (1 if i < units - base * nt else 0)
        tiles.append((t0, u * 16))
        t0 += u * 16
    assert t0 == K
    return tiles


def _build_graph_raw(K, c0, c1, c2):
    import concourse.bass as bass  # noqa: F401
    from contextlib import ExitStack
    from concourse import bacc, mybir

    f8 = mybir.dt.float8e4
    bf = mybir.dt.bfloat16
    f32 = mybir.dt.float32
    AF = mybir.ActivationFunctionType
    DR = mybir.MatmulPerfMode.DoubleRow
    wdt = [f8, f8, f8, bf]

    nc = bacc.Bacc("TRN2", target_bir_lowering=False, debug=False,
                   num_devices=NCORES)

    tiles_pre = _token_tiles(K)
    nt_pre = len(tiles_pre)
    xT_d = nc.declare_dram_parameter("xT", [nt_pre, P, 8, TT], f8, False)
    w_d = []
    for li, (ib, ob) in enumerate(_LAYER_BLOCKS):
        ocols = _WBLK_OCOLS[li]
        nblk = (ob * P) // ocols if li < 3 else 1
        w_d.append(nc.declare_dram_parameter(
            f"w{li}", [nblk, P, ib, ocols], wdt[li], False))
    out_d = nc.declare_dram_parameter("out", [1, K], f32, True)

    tiles = _token_tiles(K)
    nt = len(tiles)
    wnblk = [(ob * P) // _WBLK_OCOLS[li] if li < 3 else 1
             for li, (ib, ob) in enumerate(_LAYER_BLOCKS)]

    # ---- group schedule (python-side bookkeeping) ----
    # groups: (tile, layer, o); relu engine alternates per o
    order = []
    for ti in range(nt):
        for li in range(3):
            ob = _LAYER_BLOCKS[li][1]
            for o in range(ob):
                order.append((ti, li, o))
                # L3 of the previous tile slots in after two L0 groups of
                # this tile, giving its h3 relus time to land
                if li == 0 and o == 1 and ti > 0:
                    order.append((ti - 1, 3, 0))
    order.append((nt - 1, 3, 0))
    groups = []
    act_cnt = dve_cnt = 0
    for (ti, li, o) in order:
        if li == 3 or o % 2 == 0:
            act_cnt += 1
            eng, cnt = "act", act_cnt
        else:
            dve_cnt += 1
            eng, cnt = "dve", dve_cnt
        groups.append(dict(ti=ti, li=li, o=o, eng=eng, cnt=cnt))
    n_groups = len(groups)
    # relu totals after each (tile, layer) for layer-boundary waits
    totals_after = {}
    a = d = 0
    for gi, g in enumerate(groups):
        if g["eng"] == "act":
            a = g["cnt"]
        else:
            d = g["cnt"]
        totals_after[(g["ti"], g["li"])] = (a, d)
    for gi, g in enumerate(groups):
        g["idx"] = gi
    by_key = {(g["ti"], g["li"], g["o"]): g for g in groups}

    with ExitStack() as ctx:
        # ---- SBUF / PSUM ----
        w0s = ctx.enter_context(nc.sbuf_tensor("w0s", [P, 8, 8, 256], f8))
        w1s = ctx.enter_context(nc.sbuf_tensor("w1s", [P, 8, 16, 128], f8))
        w2s = ctx.enter_context(nc.sbuf_tensor("w2s", [P, 8, 512], f8))
        w3s = ctx.enter_context(nc.sbuf_tensor("w3s", [P, 4, 1], bf))
        ws = [w0s, w1s, w2s, w3s]
        xs = ctx.enter_context(nc.sbuf_tensor("xs", [P, nt, 8, TT], f8))
        h1 = ctx.enter_context(nc.sbuf_tensor("h1", [P, nt, 16, TT], f8))
        h2 = ctx.enter_context(nc.sbuf_tensor("h2", [P, nt, 8, TT], f8))
        h3 = ctx.enter_context(nc.sbuf_tensor("h3", [P, nt, 4, TT], bf))
        hts = {1: h1, 2: h2, 3: h3}
        outs = ctx.enter_context(nc.sbuf_tensor("outs", [1, K], f32))
        zt = ctx.enter_context(nc.sbuf_tensor("zt", [P, 1], f32))
        banks = [ctx.enter_context(
            nc.psum_tensor("pb%d" % i, [P, TT], f32)) for i in range(8)]

        # ---- semaphores ----
        def sem(name):
            return ctx.enter_context(nc.semaphore(name))

        pe_sem = sem("pe")
        act_sem = sem("act")
        dve_sem = sem("dve")
        z_sem = sem("z")
        odma_sem = sem("odma")

        # DMA list: (key, dram_ap, sbuf_ap); emitted in this order
        dmas = []

        def add_dma(key, dst, src):
            dmas.append((key, dst, src))

        add_dma(("w", 0, 0), w0s[:, 0], w_d[0][0])
        add_dma(("x", 0, 0), xs[:, 0, 0:2, :], xT_d[0][:, 0:2, :])
        add_dma(("x", 0, 1), xs[:, 0, 2:8, :], xT_d[0][:, 2:8, :])
        for blk in range(1, wnblk[0]):
            add_dma(("w", 0, blk), w0s[:, blk], w_d[0][blk])
        for blk in range(wnblk[1]):
            add_dma(("w", 1, blk), w1s[:, blk], w_d[1][blk])
        add_dma(("w", 2, 0), w2s[:, :, :], w_d[2][0])
        add_dma(("w", 3, 0), w3s[:, :, :], w_d[3][0])
        for ti in range(1, nt):
            add_dma(("x", ti, 0), xs[:, ti], xT_d[ti])
        dma_sems = {key: sem("dma_%s" % "_".join(map(str, key)))
                    for key, _, _ in dmas}

        def wslice(li, o, k2):
            if li == 3:
                return w3s[:, k2, :]  # k2 = i block, bf16 [128,1]
            if li == 0:
                return w0s[:, o // 2, 2 * k2:2 * k2 + 2,
                           (o % 2) * P:(o % 2 + 1) * P]
            if li == 1:
                return w1s[:, o, 2 * k2:2 * k2 + 2, :]
            return w2s[:, 2 * k2:2 * k2 + 2, o * P:(o + 1) * P]

        def rhs_ap(ti, li, k2):
            t0_, tsz_ = tiles[ti]
            if li == 0:
                return xs[:, ti, 2 * k2:2 * k2 + 2, :tsz_]
            if li == 3:
                return h3[:, ti, k2, :tsz_]
            h = hts[li]
            return h[:, ti, 2 * k2:2 * k2 + 2, :tsz_]

        with nc.Block(no_gpsimd_drain=True) as block:

            @block.sync
            def _(sp):
                for key, dst, src in dmas:
                    sp.dma_start(out=dst, in_=src).then_inc(dma_sems[key], 16)
                for ti in range(nt):
                    t0_, tsz_ = tiles[ti]
                    a, _dv = totals_after[(ti, 3)]
                    sp.wait_ge(act_sem, a)
                    sp.dma_start(out=out_d[:, t0_:t0_ + tsz_],
                                 in_=outs[:, t0_:t0_ + tsz_]
                                 ).then_inc(odma_sem, 16)
                sp.wait_ge(odma_sem, 16 * nt)

            @block.tensor
            def _(pe):
                waited = {}

                def wait(s, v):
                    if waited.get(s.name, -1) < v:
                        pe.wait_ge(s, v)
                        waited[s.name] = v

                for gi, g in enumerate(groups):
                    ti, li, o = g["ti"], g["li"], g["o"]
                    t0_, tsz_ = tiles[ti]
                    npair = [4, 8, 4, 4][li]
                    # PSUM WAR: slot reused from group gi-8
                    if gi >= 8:
                        p = groups[gi - 8]
                        wait(act_sem if p["eng"] == "act" else dve_sem,
                             p["cnt"])
                    # input DMAs
                    if li == 0:
                        wait(dma_sems[("x", ti, 0)], 16)
                        if ti == 0 and gi > 0:
                            wait(dma_sems[("x", 0, 1)], 16)
                        wait(dma_sems[("w", 0, o // 2)], 16)
                    elif li == 1:
                        wait(dma_sems[("w", 1, o)], 16)
                    else:
                        wait(dma_sems[("w", li, 0)], 16)

                    ps = banks[gi % 8]
                    out_ap = (ps[0:1, :tsz_] if li == 3 else ps[:, :tsz_])
                    for k in range(npair):
                        if gi == 0 and k == 1:
                            # pairs 1-3 of tile 0 arrive in the second x DMA
                            wait(dma_sems[("x", 0, 1)], 16)
                        if li > 0:
                            # h-input RAW: wait the producing relu(s) of the
                            # blocks this pair reads (dedup keeps this cheap)
                            blocks = [k] if li == 3 else [2 * k, 2 * k + 1]
                            for bo in blocks:
                                pr = by_key[(ti, li - 1, bo)]
                                wait(act_sem if pr["eng"] == "act"
                                     else dve_sem, pr["cnt"])
                        mm = pe.matmul(
                            out_ap, wslice(li, o, k), rhs_ap(ti, li, k),
                            start=(k == 0), stop=(k == npair - 1),
                            perf_mode=(None if li == 3 else DR))
                        if k == npair - 1:
                            mm.then_inc(pe_sem, 1)

            @block.vector
            def _(dve):
                from concourse import mybir as mb
                dve.memset(zt[:], 0.0).then_inc(z_sem, 1)
                for g in groups:
                    if g["eng"] != "dve":
                        continue
                    ti, li = g["ti"], g["li"]
                    t0_, tsz_ = tiles[ti]
                    scale = [c0, c1, c2][li]
                    ps = banks[g["idx"] % 8]
                    h = hts[li + 1]
                    dve.wait_ge(pe_sem, g["idx"] + 1)
                    dve.tensor_scalar(
                        h[:, ti, g["o"], :tsz_], ps[:, :tsz_], scale, 0.0,
                        mb.AluOpType.mult, mb.AluOpType.max
                    ).then_inc(dve_sem, 1)

            @block.scalar
            def _(act):
                act.wait_ge(z_sem, 1)
                for g in groups:
                    if g["eng"] != "act":
                        continue
                    ti, li = g["ti"], g["li"]
                    t0_, tsz_ = tiles[ti]
                    ps = banks[g["idx"] % 8]
                    act.wait_ge(pe_sem, g["idx"] + 1)
                    if li == 3:
                        ins = act.activation(outs[:, t0_:t0_ + tsz_],
                                             ps[0:1, :tsz_], AF.Sigmoid,
                                             bias=zt[0:1, :])
                    else:
                        scale = [c0, c1, c2][li]
                        h = hts[li + 1]
                        ins = act.activation(h[:, ti, g["o"], :tsz_],
                                             ps[:, :tsz_], AF.Relu,
                                             bias=zt[:, :], scale=scale)
                    ins.then_inc(act_sem, 1)

    nc.finalize()
    return nc


def _build_graph(K, c0, c1, c2):
    """Build the SPMD Bass graph for capacity-K expert MLP on one core.

    c0..c2 are the descale factors folded into each layer's activation
    write (product of the input/weight pre-scales for that layer).
    """
    import concourse.bass as bass  # noqa: F401
    import concourse.tile as tile
    from concourse import bacc, mybir

    f8 = mybir.dt.float8e4
    bf = mybir.dt.bfloat16
    f32 = mybir.dt.float32
    AF = mybir.ActivationFunctionType
    DR = mybir.MatmulPerfMode.DoubleRow
    wdt = [f8, f8, f8, bf]

    nc = bacc.Bacc("TRN2", target_bir_lowering=False, debug=False,
                   num_devices=NCORES)

    xT_d = nc.declare_dram_parameter("xT", [P, 8, K], f8, False)
    # weights in o-block-major layout: [n_blocks, 128, in_blocks, blk_ocols]
    w_d = []
    for li, (ib, ob) in enumerate(_LAYER_BLOCKS):
        ocols = _WBLK_OCOLS[li]
        nblk = (ob * P) // ocols if li < 3 else 1
        w_d.append(nc.declare_dram_parameter(
            f"w{li}", [nblk, P, ib, ocols], wdt[li], False))
    out_d = nc.declare_dram_parameter("out", [1, K], f32, True)

    tiles = _token_tiles(K)
    nt = len(tiles)

    with tile.TileContext(nc) as tc:
        with (
            tc.tile_pool(name="wpool", bufs=1) as wpool,
            tc.tile_pool(name="xpool", bufs=1) as xpool,
            tc.tile_pool(name="hpool", bufs=3) as hpool,
            tc.tile_pool(name="opool", bufs=1) as opool,
            tc.tile_pool(name="psum", bufs=7, space="PSUM") as psum,
            tc.tile_pool(name="psum1", bufs=1, space="PSUM") as psum1,
        ):
            # --- DMAs, emitted in first-need order ---
            wblk = [[None] * ((ob * P) // _WBLK_OCOLS[li] if li < 3 else 1)
                    for li, (ib, ob) in enumerate(_LAYER_BLOCKS)]

            def load_wblock(li, blk):
                ib, ob = _LAYER_BLOCKS[li]
                ocols = _WBLK_OCOLS[li]
                t = wpool.tile([P, ib, ocols], wdt[li], tag=f"w{li}_{blk}",
                               name=f"w{li}_{blk}")
                nc.sync.dma_start(t[:], w_d[li][blk])
                wblk[li][blk] = t

            def wslice(li, o, k2):
                """lhsT AP for out 128-block o, DoubleRow pair k2."""
                opb = _WBLK_OCOLS[li] // P  # 128-out-blocks per dma block
                t = wblk[li][o // opb]
                off = (o % opb) * P
                return t[:, 2 * k2:2 * k2 + 2, off:off + P]

            def load_xtile(ti, split=False):
                t0, tsz = tiles[ti]
                if split:
                    # pair 0 in its own small DMA so the first matmul can
                    # start as early as possible; pairs 1-3 in one DMA
                    ta = xpool.tile([P, 2, tsz], f8, tag=f"xt_{ti}_a",
                                    name=f"x_{ti}_a")
                    nc.sync.dma_start(ta[:], xT_d[:, 0:2, t0:t0 + tsz])
                    tb = xpool.tile([P, 6, tsz], f8, tag=f"xt_{ti}_b",
                                    name=f"x_{ti}_b")
                    nc.sync.dma_start(tb[:], xT_d[:, 2:8, t0:t0 + tsz])
                    return [ta[:], tb[:, 0:2, :], tb[:, 2:4, :], tb[:, 4:6, :]]
                t = xpool.tile([P, 8, tsz], f8, tag=f"xt_{ti}", name=f"x_{ti}")
                nc.sync.dma_start(t[:], xT_d[:, :, t0:t0 + tsz])
                return [t[:, 2 * k:2 * k + 2, :] for k in range(4)]

            load_wblock(0, 0)
            xt = [load_xtile(0, split=True)]
            for li in range(4):
                for blk in range(len(wblk[li])):
                    if wblk[li][blk] is None:
                        load_wblock(li, blk)
            for ti in range(1, nt):
                xt.append(load_xtile(ti))

            def relu(dst, src, o, scale):
                # alternate engines; both apply the descale then clamp at 0
                if o % 2 == 0:
                    nc.scalar.activation(dst, src, AF.Relu, scale=scale)
                else:
                    nc.vector.tensor_scalar(dst, src, scale, 0.0,
                                            mybir.AluOpType.mult,
                                            mybir.AluOpType.max)

            outs = opool.tile([1, K], f32, tag="outs", name="outs")

            for ti, (t0, tsz) in enumerate(tiles):
                # L0: 1024 -> 2048 fp8 DoubleRow, relu
                h1 = hpool.tile([P, 16, TT], f8, tag="h1", name=f"h1_{ti}")
                for o in range(16):
                    ps = psum.tile([P, TT], f32, tag="ps", name=f"ps0_{ti}_{o}")[:, :tsz]
                    for k in range(4):
                        nc.tensor.matmul(ps, wslice(0, o, k),
                                         xt[ti][k], start=(k == 0),
                                         stop=(k == 3), perf_mode=DR)
                    relu(h1[:, o, :tsz], ps, o, c0)
                # L1: 2048 -> 1024 fp8 DoubleRow, relu
                h2 = hpool.tile([P, 8, TT], f8, tag="h2", name=f"h2_{ti}")
                for o in range(8):
                    ps = psum.tile([P, TT], f32, tag="ps", name=f"ps1_{ti}_{o}")[:, :tsz]
                    for k in range(8):
                        nc.tensor.matmul(ps, wslice(1, o, k),
                                         h1[:, 2 * k:2 * k + 2, :tsz],
                                         start=(k == 0), stop=(k == 7),
                                         perf_mode=DR)
                    relu(h2[:, o, :tsz], ps, o, c1)
                # L2: 1024 -> 512 fp8 DoubleRow, relu -> bf16
                h3 = hpool.tile([P, 4, TT], bf, tag="h3", name=f"h3_{ti}")
                for o in range(4):
                    ps = psum.tile([P, TT], f32, tag="ps", name=f"ps2_{ti}_{o}")[:, :tsz]
                    for k in range(4):
                        nc.tensor.matmul(ps, wslice(2, o, k),
                                         h2[:, 2 * k:2 * k + 2, :tsz],
                                         start=(k == 0), stop=(k == 3),
                                         perf_mode=DR)
                    relu(h3[:, o, :tsz], ps, o, c2)
                # L3: 512 -> 1 bf16, sigmoid
                ps = psum1.tile([1, TT], f32, tag="ps3", name=f"ps3_{ti}")[:, :tsz]
                for i in range(4):
                    nc.tensor.matmul(ps, wblk[3][0][:, i, :], h3[:, i, :tsz],
                                     start=(i == 0), stop=(i == 3))
                nc.scalar.activation(outs[:, t0:t0 + tsz], ps, AF.Sigmoid)

            nc.sync.dma_start(out_d[:], outs[:])

    nc.finalize()
    return nc


def _np_dt(mdt_name):
    from concourse import mybir
    return mybir.dt.np(getattr(mybir.dt, mdt_name))


def _feature_major(a2d, npdt):
    """[T, F] -> SBUF layout [128, F//128, T] (contiguous)."""
    T, F = a2d.shape
    a = np.ascontiguousarray(a2d.T.reshape(F // P, P, T).transpose(1, 0, 2))
    return a.astype(npdt)


def _weight_blocked(wg, npdt, ocols):
    """[in, out] -> [n_blocks, 128, in_blocks, ocols] contiguous."""
    fin, fout = wg.shape
    ocols = min(ocols, fout)
    # blk[ob, p, i, oc] = wg[i*128+p, ob*ocols+oc]
    a = wg.reshape(fin // P, P, fout // ocols, ocols).transpose(2, 1, 0, 3)
    return np.ascontiguousarray(a).astype(npdt)


def kernel(x, center, w0_0, w0_1, w0_2, w0_3, wc_0, wc_1, wc_2, wc_3):
    from concourse.bass_utils import run_bass_kernel_spmd

    x = np.asarray(x, dtype=np.float32)
    center = np.asarray(center, dtype=np.float32)
    w0s = [np.asarray(w, dtype=np.float32) for w in (w0_0, w0_1, w0_2, w0_3)]
    wcs = [np.asarray(w, dtype=np.float32) for w in (wc_0, wc_1, wc_2, wc_3)]

    # --- host-side router + dispatch ---
    router = np.argmax(x @ center.T, axis=1)
    idxs = [np.where(router == c)[0] for c in range(C)]
    max_cnt = max(len(ix) for ix in idxs)
    K = max(P, int(math.ceil(max_cnt / 16)) * 16)

    # gated weights per cluster, and global per-layer fp8 pre-scales
    wg = [[w0s[li] * wcs[li][c] for c in range(C)] for li in range(4)]
    FP8_MAX = 240.0
    TINY = 1e-30
    ws = [max(TINY, max(np.abs(wg[li][c]).max() for c in range(C))) / FP8_MAX
          for li in range(3)]
    hs0 = max(TINY, np.abs(x).max()) / FP8_MAX

    # estimate activation ranges on a sample to pick gains G1, G2 that keep
    # stored fp8 activations well inside the normal range
    smp = x[:: max(1, B // 512)]
    m1 = m2 = 1e-9
    for c in range(C):
        a1 = np.maximum(smp @ wg[0][c], 0)
        m1 = max(m1, a1.max())
        a2 = np.maximum(a1 @ wg[1][c], 0)
        m2 = max(m2, a2.max())
    G1 = FP8_MAX / (8.0 * m1)
    G2 = FP8_MAX / (8.0 * m2)
    c0 = float(hs0 * ws[0] * G1)
    c1 = float(ws[1] * G2 / G1)
    c2 = float(ws[2] / G2)

    use_raw = os.environ.get("KERNEL_RAW", "1") != "0"
    key = (use_raw, K, round(c0, 12), round(c1, 12), round(c2, 12))
    if key not in _graph_cache:
        builder = _build_graph_raw if use_raw else _build_graph
        _graph_cache[key] = builder(K, c0, c1, c2)
    nc = _graph_cache[key]

    f8np = _np_dt("float8e4")
    bfnp = _np_dt("bfloat16")
    in_maps = []
    for c in range(C):
        ix = idxs[c]
        xg = np.zeros((K, DIMS), np.float32)
        xg[:len(ix)] = x[ix] / hs0
        xf = _feature_major(xg, f8np)  # [128, 8, K]
        if use_raw:
            # tile-major blocked, zero-padded to TT per tile
            tls = _token_tiles(K)
            xb = np.zeros((len(tls), P, 8, TT), f8np)
            for ti, (t0, tsz) in enumerate(tls):
                xb[ti, :, :, :tsz] = xf[:, :, t0:t0 + tsz]
            m = {"xT": xb}
        else:
            m = {"xT": xf}
        for li in range(3):
            m[f"w{li}"] = _weight_blocked(wg[li][c] / ws[li], f8np, _WBLK_OCOLS[li])
        m["w3"] = _weight_blocked(wg[3][c], bfnp, _WBLK_OCOLS[3])
        in_maps.append(m)

    try:
        res = run_bass_kernel_spmd(nc, in_maps, core_ids=list(range(NCORES)))
    except ModuleNotFoundError:
        # Axon stub without the NTFF profile hook: retry without tracing.
        os.environ["BASS_NEVER_TRACE"] = "1"
        res = run_bass_kernel_spmd(nc, in_maps, core_ids=list(range(NCORES)))
    global last_run
    last_run = res

    out = np.zeros(B, np.float32)
    for c in range(C):
        ix = idxs[c]
        out[ix] = res.results[c]["out"][0, :len(ix)]
    return out

